# revision 11
# baseline (speedup 1.0000x reference)
"""EpisodicSlotWriter Trainium2 Bass kernel.

Data-parallel over batch: B=1024 rows split across 8 NeuronCores
(128 rows/core = 128 SBUF partitions, one batch row per partition).

Per core:
  - stream epi_keys in K-chunks HBM->SBUF, computing per-slot dot(ek, wk_n)
    (DVE scalar_tensor_tensor + accum) and per-slot sumsq (ACT Square+accum)
    on the fly, then write each chunk back out unchanged (bulk copy of
    epi_keys_new).
  - epi_vals is a pure passthrough copy (load + store).
  - sim = dot * 1/(||ek||+1e-6); argmax via DVE max/max_index.
  - LRU score from epi_age/epi_strength (bitwise-faithful to the reference
    expression so the argmax matches exactly).
  - gather the selected slot row of epi_keys/epi_vals with indirect DMA,
    blend with write_key/write_val, renormalize keys, scatter back into the
    outputs (after the bulk copies, enforced with explicit dep edges).
"""

import sys

if "/opt/trn_rl_repo" not in sys.path:
    sys.path.insert(0, "/opt/trn_rl_repo")

import numpy as np

B, K, D = 1024, 128, 512
M = 8               # cores
BL = B // M         # 128 rows per core
KC = 16             # slots per chunk
NCH = K // KC       # 8 chunks

MERGE_THR = 0.85
MIN_STR = 0.001
STR_DECAY = 0.999
WRITE_ALPHA = 0.25  # == EPI_WRITE_BETA in the reference

_cache = {}


def _split_waits(nc, mybir, maxw=1):
    """walrus in this env rejects >1 sync wait on one instruction; hoist
    extra waits onto preceding same-engine NoOps."""
    nid = 0
    for f in nc.m.functions:
        for bb in f.blocks:
            out, changed = [], False
            for ins in bb.instructions:
                si = ins.sync_info
                if si is not None and si.on_wait and len(si.on_wait) > maxw:
                    waits = list(si.on_wait)
                    head, tail = waits[:-maxw], waits[-maxw:]
                    for i in range(0, len(head), maxw):
                        nop = mybir.InstNoOp(name=f"wsplit-{nid}", ins=[], outs=[])
                        nid += 1
                        nop.engine = ins.engine
                        nop.sync_info = mybir.SyncInfo(
                            on_wait=head[i : i + maxw], on_update=[]
                        )
                        out.append(nop)
                    ins.sync_info = mybir.SyncInfo(
                        on_wait=tail, on_update=list(si.on_update)
                    )
                    changed = True
                out.append(ins)
            if changed:
                bb.instructions = out


def _build(split=True, repeat=1):
    import concourse.bass as bass
    import concourse.tile as tile
    from concourse import mybir
    from concourse.bass import IndirectOffsetOnAxis
    from concourse.tile import add_dep_helper

    f32 = mybir.dt.float32
    i32 = mybir.dt.int32
    u32 = mybir.dt.uint32
    Alu = mybir.AluOpType
    Act = mybir.ActivationFunctionType

    nc = bass.Bass("TRN2", target_bir_lowering=False, debug=False)

    wk_d = nc.dram_tensor("write_key", [BL, D], f32, kind="ExternalInput")
    wv_d = nc.dram_tensor("write_val", [BL, D], f32, kind="ExternalInput")
    ws_d = nc.dram_tensor("write_strength", [BL, 1], f32, kind="ExternalInput")
    ek_d = nc.dram_tensor("epi_keys", [BL, K, D], f32, kind="ExternalInput")
    ev_d = nc.dram_tensor("epi_vals", [BL, K, D], f32, kind="ExternalInput")
    age_d = nc.dram_tensor("epi_age", [BL, K], f32, kind="ExternalInput")
    stg_d = nc.dram_tensor("epi_strength", [BL, K], f32, kind="ExternalInput")

    eko_d = nc.dram_tensor("ek_out", [BL, K, D], f32, kind="ExternalOutput")
    evo_d = nc.dram_tensor("ev_out", [BL, K, D], f32, kind="ExternalOutput")
    ageo_d = nc.dram_tensor("age_out", [BL, K], f32, kind="ExternalOutput")
    stgo_d = nc.dram_tensor("stg_out", [BL, K], f32, kind="ExternalOutput")
    slot_d = nc.dram_tensor("slot_out", [BL, 1], i32, kind="ExternalOutput")
    sim_d = nc.dram_tensor("sim_out", [BL, 1], f32, kind="ExternalOutput")

    ek2d = ek_d.ap().rearrange("b k d -> (b k) d")
    ev2d = ev_d.ap().rearrange("b k d -> (b k) d")
    eko2d = eko_d.ap().rearrange("b k d -> (b k) d")
    evo2d = evo_d.ap().rearrange("b k d -> (b k) d")

    with tile.TileContext(nc) as tc:
        with (
            tc.tile_pool(name="keys", bufs=3) as kpool,
            tc.tile_pool(name="vals", bufs=2) as vpool,
            tc.tile_pool(name="small", bufs=1) as sp,
        ):
          # `repeat` re-emits the whole body (timing amplification only;
          # repeat=1 for the real kernel).
          for _rep in range(repeat):
            # ---- small loads -------------------------------------------------
            wk_t = sp.tile([BL, D], f32, tag="wk")
            nc.sync.dma_start(wk_t[:], wk_d.ap())
            wv_t = sp.tile([BL, D], f32, tag="wv")
            nc.sync.dma_start(wv_t[:], wv_d.ap())
            ws_t = sp.tile([BL, 1], f32, tag="ws")
            nc.sync.dma_start(ws_t[:], ws_d.ap())
            age_t = sp.tile([BL, K], f32, tag="age")
            nc.sync.dma_start(age_t[:], age_d.ap())
            stg_t = sp.tile([BL, K], f32, tag="stg")
            nc.sync.dma_start(stg_t[:], stg_d.ap())

            # iotas (gpsimd compute, once)
            iota_row_i = sp.tile([BL, K], i32, tag="iri")
            nc.gpsimd.iota(iota_row_i[:], pattern=[[1, K]], base=0, channel_multiplier=0)
            rowbase_i = sp.tile([BL, 1], i32, tag="rbi")
            nc.gpsimd.iota(rowbase_i[:], pattern=[[1, 1]], base=0, channel_multiplier=K)
            iota_row_f = sp.tile([BL, K], f32, tag="irf")
            nc.vector.tensor_copy(iota_row_f[:], iota_row_i[:])

            # ---- normalize write_key ----------------------------------------
            scr_wk = sp.tile([BL, D], f32, tag="scrwk")
            wk_ss = sp.tile([BL, 1], f32, tag="wkss")
            nc.scalar.activation(scr_wk[:], wk_t[:], Act.Square, accum_out=wk_ss[:])
            wk_nrm = sp.tile([BL, 1], f32, tag="wknrm")
            nc.scalar.activation(wk_nrm[:], wk_ss[:], Act.Sqrt)
            wk_nrm_e = sp.tile([BL, 1], f32, tag="wknrme")
            nc.vector.tensor_scalar(
                out=wk_nrm_e[:], in0=wk_nrm[:], scalar1=1e-6, scalar2=None, op0=Alu.add
            )
            wk_rec = sp.tile([BL, 1], f32, tag="wkrec")
            nc.vector.reciprocal(wk_rec[:], wk_nrm_e[:])
            wk_n = sp.tile([BL, D], f32, tag="wkn")
            nc.vector.tensor_scalar(
                out=wk_n[:], in0=wk_t[:], scalar1=wk_rec[:], scalar2=None, op0=Alu.mult
            )

            # ---- stream epi_keys / epi_vals ---------------------------------
            dots = sp.tile([BL, K], f32, tag="dots")
            ssq = sp.tile([BL, K], f32, tag="ssq")
            scr_dve = sp.tile([BL, D], f32, tag="scrdve")
            scr_act = sp.tile([BL, D], f32, tag="scract")

            kstores, vstores = [], []
            for ci in range(NCH):
                k0 = ci * KC
                kt = kpool.tile([BL, KC * D], f32, tag="kt")
                nc.sync.dma_start(kt[:], ek_d.ap()[:, k0 : k0 + KC, :])
                vt = vpool.tile([BL, KC * D], f32, tag="vt")
                nc.sync.dma_start(vt[:], ev_d.ap()[:, k0 : k0 + KC, :])

                for j in range(KC):
                    k = k0 + j
                    sl = kt[:, j * D : (j + 1) * D]
                    nc.vector.scalar_tensor_tensor(
                        out=scr_dve[:], in0=sl, scalar=0.0, in1=wk_n[:],
                        op0=Alu.bypass, op1=Alu.mult, accum_out=dots[:, k : k + 1],
                    )
                    nc.scalar.activation(
                        scr_act[:], sl, Act.Square, accum_out=ssq[:, k : k + 1]
                    )

                ks = nc.gpsimd.dma_start(eko_d.ap()[:, k0 : k0 + KC, :], kt[:])
                kstores.append(ks)
                vs = nc.gpsimd.dma_start(evo_d.ap()[:, k0 : k0 + KC, :], vt[:])
                vstores.append(vs)

            # ---- sim + argmax -----------------------------------------------
            nrm_e = sp.tile([BL, K], f32, tag="nrme")
            nc.scalar.activation(nrm_e[:], ssq[:], Act.Sqrt)
            nc.vector.tensor_scalar(
                out=nrm_e[:], in0=nrm_e[:], scalar1=1e-6, scalar2=None, op0=Alu.add
            )
            rec_e = sp.tile([BL, K], f32, tag="rece")
            nc.vector.reciprocal(rec_e[:], nrm_e[:])
            sim = sp.tile([BL, K], f32, tag="sim")
            nc.vector.tensor_tensor(out=sim[:], in0=dots[:], in1=rec_e[:], op=Alu.mult)

            max8 = sp.tile([BL, 8], f32, tag="max8")
            idx8 = sp.tile([BL, 8], u32, tag="idx8")
            nc.vector.max(max8[:], sim[:])
            nc.vector.max_index(idx8[:], max8[:], sim[:])
            best_sim = max8[:, 0:1]
            best_idx_u = idx8[:, 0:1]

            # ---- LRU score (bitwise-faithful) -------------------------------
            stg_c = sp.tile([BL, K], f32, tag="stgc")
            nc.vector.tensor_scalar(
                out=stg_c[:], in0=stg_t[:], scalar1=0.0, scalar2=1.0,
                op0=Alu.max, op1=Alu.min,
            )
            onem_s = sp.tile([BL, K], f32, tag="onems")
            nc.vector.tensor_scalar(
                out=onem_s[:], in0=stg_c[:], scalar1=-1.0, scalar2=1.0,
                op0=Alu.mult, op1=Alu.add,
            )
            score = sp.tile([BL, K], f32, tag="score")
            nc.vector.scalar_tensor_tensor(
                out=score[:], in0=onem_s[:], scalar=0.01, in1=age_t[:],
                op0=Alu.mult, op1=Alu.add,
            )
            amax8 = sp.tile([BL, 8], f32, tag="amax8")
            aidx8 = sp.tile([BL, 8], u32, tag="aidx8")
            nc.vector.max(amax8[:], score[:])
            nc.vector.max_index(aidx8[:], amax8[:], score[:])
            lru_idx_u = aidx8[:, 0:1]

            # ---- slot select -------------------------------------------------
            mask = sp.tile([BL, 1], f32, tag="mask")
            nc.vector.tensor_scalar(
                out=mask[:], in0=best_sim, scalar1=MERGE_THR, scalar2=None, op0=Alu.is_gt
            )
            best_f = sp.tile([BL, 1], f32, tag="bestf")
            nc.vector.tensor_copy(best_f[:], best_idx_u)
            lru_f = sp.tile([BL, 1], f32, tag="lruf")
            nc.vector.tensor_copy(lru_f[:], lru_idx_u)
            mb = sp.tile([BL, 1], f32, tag="mb")
            nc.vector.tensor_tensor(out=mb[:], in0=mask[:], in1=best_f[:], op=Alu.mult)
            notmask1 = sp.tile([BL, 1], f32, tag="notmask1")
            nc.vector.tensor_scalar(
                out=notmask1[:], in0=mask[:], scalar1=-1.0, scalar2=1.0,
                op0=Alu.mult, op1=Alu.add,
            )
            ml = sp.tile([BL, 1], f32, tag="ml")
            nc.vector.tensor_tensor(out=ml[:], in0=notmask1[:], in1=lru_f[:], op=Alu.mult)
            slot_f = sp.tile([BL, 1], f32, tag="slotf")
            nc.vector.tensor_tensor(out=slot_f[:], in0=mb[:], in1=ml[:], op=Alu.add)
            slot_i = sp.tile([BL, 1], i32, tag="sloti")
            nc.vector.tensor_copy(slot_i[:], slot_f[:])

            # ---- onehot, age, strength --------------------------------------
            onehot = sp.tile([BL, K], f32, tag="onehot")
            nc.vector.tensor_scalar(
                out=onehot[:], in0=iota_row_f[:], scalar1=slot_f[:], scalar2=None,
                op0=Alu.is_equal,
            )
            notmask = sp.tile([BL, K], f32, tag="notmask")
            nc.vector.tensor_scalar(
                out=notmask[:], in0=onehot[:], scalar1=-1.0, scalar2=1.0,
                op0=Alu.mult, op1=Alu.add,
            )
            age_new = sp.tile([BL, K], f32, tag="agenew")
            nc.vector.scalar_tensor_tensor(
                out=age_new[:], in0=age_t[:], scalar=1.0, in1=notmask[:],
                op0=Alu.add, op1=Alu.mult,
            )

            s_dec = sp.tile([BL, K], f32, tag="sdec")
            nc.vector.tensor_scalar(
                out=s_dec[:], in0=stg_t[:], scalar1=STR_DECAY, scalar2=None, op0=Alu.mult
            )
            scr_k = sp.tile([BL, K], f32, tag="scrk")
            prev = sp.tile([BL, 1], f32, tag="prev")
            nc.vector.scalar_tensor_tensor(
                out=scr_k[:], in0=s_dec[:], scalar=0.0, in1=onehot[:],
                op0=Alu.bypass, op1=Alu.mult, accum_out=prev[:],
            )
            ws_c = sp.tile([BL, 1], f32, tag="wsc")
            nc.vector.tensor_scalar(
                out=ws_c[:], in0=ws_t[:], scalar1=0.0, scalar2=1.0,
                op0=Alu.max, op1=Alu.min,
            )
            onem_p = sp.tile([BL, 1], f32, tag="onemp")
            nc.vector.tensor_scalar(
                out=onem_p[:], in0=prev[:], scalar1=-1.0, scalar2=1.0,
                op0=Alu.mult, op1=Alu.add,
            )
            delta = sp.tile([BL, 1], f32, tag="delta")
            nc.vector.tensor_tensor(out=delta[:], in0=ws_c[:], in1=onem_p[:], op=Alu.mult)
            upd0 = sp.tile([BL, 1], f32, tag="upd0")
            nc.vector.tensor_tensor(out=upd0[:], in0=prev[:], in1=delta[:], op=Alu.add)
            upd = sp.tile([BL, 1], f32, tag="upd")
            nc.vector.tensor_scalar(
                out=upd[:], in0=upd0[:], scalar1=MIN_STR, scalar2=1.0,
                op0=Alu.max, op1=Alu.min,
            )
            su = sp.tile([BL, K], f32, tag="su")
            nc.vector.tensor_scalar(
                out=su[:], in0=onehot[:], scalar1=upd[:], scalar2=None, op0=Alu.mult
            )
            s_keep = sp.tile([BL, K], f32, tag="skeep")
            nc.vector.tensor_tensor(out=s_keep[:], in0=s_dec[:], in1=notmask[:], op=Alu.mult)
            stg_new = sp.tile([BL, K], f32, tag="stgnew")
            nc.vector.tensor_tensor(out=stg_new[:], in0=s_keep[:], in1=su[:], op=Alu.add)

            # ---- gather selected rows, blend, renorm ------------------------
            offs_i = sp.tile([BL, 1], i32, tag="offsi")
            nc.vector.tensor_tensor(
                out=offs_i[:], in0=rowbase_i[:], in1=slot_i[:], op=Alu.add
            )
            old_k = sp.tile([BL, D], f32, tag="oldk")
            nc.gpsimd.indirect_dma_start(
                out=old_k[:], out_offset=None, in_=ek2d,
                in_offset=IndirectOffsetOnAxis(ap=offs_i[:, 0:1], axis=0),
            )
            old_v = sp.tile([BL, D], f32, tag="oldv")
            nc.gpsimd.indirect_dma_start(
                out=old_v[:], out_offset=None, in_=ev2d,
                in_offset=IndirectOffsetOnAxis(ap=offs_i[:, 0:1], axis=0),
            )

            alpha = sp.tile([BL, 1], f32, tag="alpha")
            nc.vector.tensor_scalar(
                out=alpha[:], in0=ws_c[:], scalar1=WRITE_ALPHA, scalar2=None, op0=Alu.mult
            )
            onem_a = sp.tile([BL, 1], f32, tag="onema")
            nc.vector.tensor_scalar(
                out=onem_a[:], in0=alpha[:], scalar1=-1.0, scalar2=1.0,
                op0=Alu.mult, op1=Alu.add,
            )
            t1 = sp.tile([BL, D], f32, tag="t1")
            nc.vector.tensor_scalar(
                out=t1[:], in0=old_k[:], scalar1=onem_a[:], scalar2=None, op0=Alu.mult
            )
            t2 = sp.tile([BL, D], f32, tag="t2")
            nc.vector.tensor_scalar(
                out=t2[:], in0=wk_t[:], scalar1=alpha[:], scalar2=None, op0=Alu.mult
            )
            nk0 = sp.tile([BL, D], f32, tag="nk0")
            nc.vector.tensor_tensor(out=nk0[:], in0=t1[:], in1=t2[:], op=Alu.add)

            scr_nk = sp.tile([BL, D], f32, tag="scrnk")
            nk_ss = sp.tile([BL, 1], f32, tag="nkss")
            nc.scalar.activation(scr_nk[:], nk0[:], Act.Square, accum_out=nk_ss[:])
            nk_nrm = sp.tile([BL, 1], f32, tag="nknrm")
            nc.scalar.activation(nk_nrm[:], nk_ss[:], Act.Sqrt)
            nc.vector.tensor_scalar(
                out=nk_nrm[:], in0=nk_nrm[:], scalar1=1e-6, scalar2=None, op0=Alu.add
            )
            nk_rec = sp.tile([BL, 1], f32, tag="nkrec")
            nc.vector.reciprocal(nk_rec[:], nk_nrm[:])
            new_k = sp.tile([BL, D], f32, tag="newk")
            nc.vector.tensor_scalar(
                out=new_k[:], in0=nk0[:], scalar1=nk_rec[:], scalar2=None, op0=Alu.mult
            )

            t3 = sp.tile([BL, D], f32, tag="t3")
            nc.vector.tensor_scalar(
                out=t3[:], in0=old_v[:], scalar1=onem_a[:], scalar2=None, op0=Alu.mult
            )
            t4 = sp.tile([BL, D], f32, tag="t4")
            nc.vector.tensor_scalar(
                out=t4[:], in0=wv_t[:], scalar1=alpha[:], scalar2=None, op0=Alu.mult
            )
            new_v = sp.tile([BL, D], f32, tag="newv")
            nc.vector.tensor_tensor(out=new_v[:], in0=t3[:], in1=t4[:], op=Alu.add)

            # ---- small outputs ----------------------------------------------
            nc.gpsimd.dma_start(ageo_d.ap(), age_new[:])
            nc.gpsimd.dma_start(stgo_d.ap(), stg_new[:])
            nc.gpsimd.dma_start(slot_d.ap(), slot_i[:])
            nc.gpsimd.dma_start(sim_d.ap(), best_sim)

            # ---- scatter (must land after the bulk copies) ------------------
            sc_k = nc.gpsimd.indirect_dma_start(
                out=eko2d,
                out_offset=IndirectOffsetOnAxis(ap=offs_i[:, 0:1], axis=0),
                in_=new_k[:], in_offset=None,
            )
            sc_v = nc.gpsimd.indirect_dma_start(
                out=evo2d,
                out_offset=IndirectOffsetOnAxis(ap=offs_i[:, 0:1], axis=0),
                in_=new_v[:], in_offset=None,
            )
            for st in kstores:
                add_dep_helper(sc_k.ins, st.ins, reason="scatter after bulk keys copy")
            for st in vstores:
                add_dep_helper(sc_v.ins, st.ins, reason="scatter after bulk vals copy")

    nc.finalize()
    if split:
        _split_waits(nc, mybir)
    return nc


def _get_nc():
    if "nc" not in _cache:
        _cache["nc"] = _build()
    return _cache["nc"]


def kernel(**inputs):
    from concourse.bass_utils import run_bass_kernel_spmd

    names = [
        "write_key", "write_val", "write_strength",
        "epi_keys", "epi_vals", "epi_age", "epi_strength",
    ]
    arrs = {n: np.ascontiguousarray(np.asarray(inputs[n], dtype=np.float32)) for n in names}

    nc = _get_nc()
    in_maps = [
        {n: arrs[n][c * BL : (c + 1) * BL] for n in names} for c in range(M)
    ]
    res = run_bass_kernel_spmd(nc, in_maps, list(range(M)))
    r = res.results

    ek_new = np.concatenate([r[c]["ek_out"] for c in range(M)], axis=0)
    ev_new = np.concatenate([r[c]["ev_out"] for c in range(M)], axis=0)
    age_new = np.concatenate([r[c]["age_out"] for c in range(M)], axis=0)
    stg_new = np.concatenate([r[c]["stg_out"] for c in range(M)], axis=0)
    slot_idx = np.concatenate(
        [r[c]["slot_out"].reshape(BL) for c in range(M)], axis=0
    ).astype(np.int32)
    best_sim = np.concatenate(
        [r[c]["sim_out"].reshape(BL) for c in range(M)], axis=0
    ).astype(np.float32)
    return ek_new, ev_new, age_new, stg_new, slot_idx, best_sim
